# revision 1
# baseline (speedup 1.0000x reference)
"""Dependency-GCN via dma_scatter_add for 8 Trainium2 NeuronCores.

Strategy (single SPMD program, no collectives):
  - Each core owns a contiguous range of 3750 destination nodes; edges
    are routed to their dst-owner core (fwd: dep, rev: gov).
  - Host pre-combines edges sharing (direction, relation, dst): their
    source rows are summed into extra rows of a per-core compacted
    feature table x_ext (gather sources re-indexed, < 32767 so int16
    gather indices work).  After combining, each (direction, relation)
    group has at most ONE edge per dst.  Rows are stored as
    interleaved fp8 pairs (e5m2 value, e4m3 residual) so one 512-byte
    gather feeds an error-compensated fp8 DoubleRow transform:
    msg = x8@W8 + x8@Wr + xr@W8 at half the fp16 PE cost with
    near-fp16 accuracy (wide-exponent e5m2 keeps residuals normal).
  - dma_scatter_add loses updates for duplicate indices WITHIN one
    instruction (measured on HW) but separate instructions on a queue
    serialize; the (dir, rel) grouping makes every piece duplicate-
    free by construction.  To overlap the per-piece WAW chains, dst
    rows are split into two contiguous halves of the out tensor (each
    with its own trash row for pads) scattered on two different SWDGE
    queues: ranges are disjoint, so the two chains pipeline.
  - Device pipeline per 128-edge block: transpose-gather source token
    rows (7-block pieces; the transposer FIFO caps ~1000 indices per
    instruction) -> three DoubleRow matmuls into paired PSUM banks ->
    one copy per 2 blocks to fp16 SBUF piece buffers (alternating
    Activation/DVE) -> per piece: dma_scatter_add into the HBM out
    accumulator rows (out[dst] += msg, fp16 accumulate).
  - out [2*1876, 256] fp16 (A rows 0..1874, A trash, B rows, B trash)
    is initialized by a DRAM->DRAM DMA from a host-precomputed bias
    image (b_self + indeg_r @ b_fwd/b_rev); the self transform rides
    the pipeline as relation 20 (unique dsts).
"""

import sys

if "/opt/trn_rl_repo" not in sys.path:
    sys.path.insert(0, "/opt/trn_rl_repo")

import os as _os
import numpy as np

import concourse.bacc as bacc
import concourse.mybir as mybir
from concourse.tile import TileContext
from concourse.bass_utils import run_bass_kernel_spmd

F32 = mybir.dt.float32
F16 = mybir.dt.float16
F8E4 = mybir.dt.float8e4
F8E5 = mybir.dt.float8e5
I16 = mybir.dt.int16
NP8E4 = mybir.dt.np(F8E4)
NP8E5 = mybir.dt.np(F8E5)
DR = mybir.MatmulPerfMode.DoubleRow

N_NODES = 30000
N_REL = 10
D = 256
N_CORES = 8
NODES_PER_CORE = N_NODES // N_CORES          # 3750
HALF = NODES_PER_CORE // 2                    # 1875 rows per half
HROWS = HALF + 1                              # + trash row
SELF_REL = 20
GB = int(_os.environ.get("GCN_GB", "7"))     # blocks per gather piece


# ---------------------------------------------------------------- host prep

def _pack_idx16(idx: np.ndarray) -> np.ndarray:
    """[n] -> [128, n//16] int16: idx i at (partition i%16, col i//16), x8."""
    n = idx.shape[0]
    t = idx.astype(np.int16).reshape(n // 16, 16).T
    return np.tile(t, (8, 1))


def _split8(rows32: np.ndarray) -> np.ndarray:
    """[n, 256] fp32 -> [n, 512] bytes: col 2i = e5m2(x_i) (wide exponent
    range: the residual of the residual never goes subnormal), 2i+1 =
    e4m3(x_i - e5m2(x_i))."""
    hi = rows32.astype(NP8E5)
    lo = (rows32 - hi.astype(np.float32)).astype(NP8E4)
    out = np.empty((rows32.shape[0], 2 * rows32.shape[1]), np.uint8)
    out[:, 0::2] = hi.view(np.uint8)
    out[:, 1::2] = lo.view(np.uint8)
    return out


def prepare(x, W_self, b_self, W_fwd, b_fwd, W_rev, b_rev,
            dep_idx, rel_idx, gov_idx):
    dep_idx = np.asarray(dep_idx).astype(np.int64)
    rel_idx = np.asarray(rel_idx).astype(np.int64)
    gov_idx = np.asarray(gov_idx).astype(np.int64)
    x = np.asarray(x, np.float32)
    x16 = x.astype(np.float16)

    # weight stack [128, 2, 21, 2, 256] fp8 bytes:
    #   kind 0 = e5m2(W), kind 1 = e5m2(W - W8)
    # msg = x8@W8 + x8@Wr + xr@W8 with x8 e5m2 / xr e4m3 (see _split8);
    # the xr term mixes e4m3 lhsT with the e5m2 W8 rhs (verified on HW).
    W_all = np.zeros((21, D, D), np.float32)
    W_all[0:10] = np.asarray(W_fwd, np.float32)
    W_all[10:20] = np.asarray(W_rev, np.float32)
    W_all[20] = np.asarray(W_self, np.float32)
    W8 = W_all.astype(NP8E5)
    Wr = (W_all - W8.astype(np.float32)).astype(NP8E5)
    wsb = np.zeros((128, 2, 21, 2, D), np.uint8)
    for rw in range(21):
        for h in range(2):
            sl = slice(h * 128, (h + 1) * 128)
            wsb[:, 0, rw, h, :] = W8[rw, sl].view(np.uint8)
            wsb[:, 1, rw, h, :] = Wr[rw, sl].view(np.uint8)

    # ---- per-core edges keyed by (relW, local dst); dedupe cells
    core_key = [[] for _ in range(N_CORES)]
    core_src = [[] for _ in range(N_CORES)]
    for d in range(2):
        if d == 0:
            src_a, dst_a, relw_a = gov_idx, dep_idx, rel_idx
        else:
            src_a, dst_a, relw_a = dep_idx, gov_idx, rel_idx + 10
        core_of = dst_a // NODES_PER_CORE
        for c in range(N_CORES):
            m = core_of == c
            core_key[c].append(relw_a[m] * NODES_PER_CORE
                               + (dst_a[m] - c * NODES_PER_CORE))
            core_src[c].append(src_a[m])

    per_core = []
    max_cells = np.zeros((20, 2), np.int64)   # per (relW, half) over cores
    for c in range(N_CORES):
        key = np.concatenate(core_key[c])
        src = np.concatenate(core_src[c])
        order = np.argsort(key, kind="stable")
        key, src = key[order], src[order]
        ukey, start, cnt = np.unique(key, return_index=True,
                                     return_counts=True)
        n_u = ukey.shape[0]
        single = cnt == 1
        multi = np.nonzero(~single)[0]
        comb_rows = np.zeros((len(multi), D), np.float32)
        for j, ui in enumerate(multi):
            s = start[ui]
            comb_rows[j] = x[src[s:s + cnt[ui]]].sum(0)
        used = np.unique(src[start[single]])
        remap = np.full(N_NODES, -1, np.int64)
        remap[used] = np.arange(len(used))
        nx0 = len(used)
        gsrc = np.empty(n_u, np.int64)
        gsrc[single] = remap[src[start[single]]]
        gsrc[~single] = nx0 + np.arange(len(multi))
        x_ext = _split8(np.concatenate([x[used], comb_rows], axis=0))
        relw = ukey // NODES_PER_CORE
        dstl = ukey % NODES_PER_CORE
        half = (dstl >= HALF).astype(np.int64)
        cells = {}
        for rw in range(20):
            for h in range(2):
                m = (relw == rw) & (half == h)
                cells[(rw, h)] = (dstl[m] - h * HALF, gsrc[m])
                max_cells[rw, h] = max(max_cells[rw, h], int(m.sum()))
        per_core.append((cells, x_ext))

    # schedule: pieces (relW, half, nblocks); self placement via env
    nblk_self = (HALF + 127) // 128               # 15
    selfp = [(SELF_REL, 0, nblk_self), (SELF_REL, 1, nblk_self)]
    relp = []
    for rw in range(20):
        for h in range(2):
            nb = (int(max_cells[rw, h]) + 127) // 128
            if nb > 0:
                relp.append((rw, h, nb))
    pos = _os.environ.get("GCN_SELFPOS", "first")
    if pos == "first":
        pieces = selfp + relp
    elif pos == "last":
        pieces = relp + selfp
    else:  # mid
        k = int(pos)
        pieces = relp[:k] + selfp + relp[k:]
    nblk_total = sum(p[2] for p in pieces)

    nx_max = max(pc[1].shape[0] for pc in per_core)
    nx_max = (nx_max + 15) // 16 * 16
    self_base = nx_max
    nx_total = nx_max + NODES_PER_CORE
    assert nx_total < 32768, nx_total

    in_maps = []
    for c in range(N_CORES):
        cells, x_ext = per_core[c]
        xb = np.zeros((nx_total, 2 * D), np.uint8)  # int16 token pairs
        xb[:x_ext.shape[0]] = x_ext
        xb[self_base:self_base + NODES_PER_CORE] = \
            _split8(x[c * NODES_PER_CORE:(c + 1) * NODES_PER_CORE])

        gidx = np.zeros(nblk_total * 128, np.int16)
        sidx = np.full(nblk_total * 128, HALF, np.int16)  # trash (local)
        bi = 0
        for (rw, h, nb) in pieces:
            s0 = bi * 128
            if rw == SELF_REL:
                n_real = HALF
                sidx[s0:s0 + n_real] = np.arange(HALF, dtype=np.int16)
                gidx[s0:s0 + n_real] = (
                    self_base + h * HALF
                    + np.arange(HALF)).astype(np.int16)
            else:
                dstl, gs = cells[(rw, h)]
                n_real = dstl.shape[0]
                sidx[s0:s0 + n_real] = dstl.astype(np.int16)
                gidx[s0:s0 + n_real] = gs.astype(np.int16)
            bi += nb
        assert bi == nblk_total

        # bias image [2*HROWS, 256] fp16 in half-local layout
        lo = c * NODES_PER_CORE
        hi = lo + NODES_PER_CORE
        cnt_f = np.zeros((NODES_PER_CORE, N_REL), np.float32)
        mf = (dep_idx >= lo) & (dep_idx < hi)
        np.add.at(cnt_f, (dep_idx[mf] - lo, rel_idx[mf]), 1.0)
        cnt_r = np.zeros((NODES_PER_CORE, N_REL), np.float32)
        mr = (gov_idx >= lo) & (gov_idx < hi)
        np.add.at(cnt_r, (gov_idx[mr] - lo, rel_idx[mr]), 1.0)
        bias = (np.asarray(b_self, np.float32)[None, :]
                + cnt_f @ np.asarray(b_fwd, np.float32)
                + cnt_r @ np.asarray(b_rev, np.float32))
        binit = np.zeros((2 * HROWS, D), np.float32)
        binit[0:HALF] = bias[0:HALF]
        binit[HROWS:HROWS + HALF] = bias[HALF:]
        in_maps.append({
            "x_ext": xb.view(np.int16),
            "wsb": wsb,
            "gidx": _pack_idx16(gidx),
            "sidx": _pack_idx16(sidx),
            "binit": binit.astype(np.float16),
        })

    return pieces, nblk_total, nx_total, in_maps


# ---------------------------------------------------------------- device

def build_bass(pieces, nblk_total, nx_total):
    nc = bacc.Bacc(num_swdge_queues=3)
    x_ext = nc.declare_dram_parameter("x_ext", [nx_total, D], I16,
                                      isOutput=False)
    wsb = nc.declare_dram_parameter("wsb", [128, 2, 21, 2, D], F8E5,
                                    isOutput=False)
    gidx = nc.declare_dram_parameter("gidx", [128, nblk_total * 8], I16,
                                     isOutput=False)
    sidx = nc.declare_dram_parameter("sidx", [128, nblk_total * 8], I16,
                                     isOutput=False)
    binit = nc.declare_dram_parameter("binit", [2 * HROWS, D], F16,
                                      isOutput=False)
    out = nc.declare_dram_parameter("out", [2 * HROWS, D], F16,
                                    isOutput=True)

    with TileContext(nc) as tc:
        with (
            tc.tile_pool(name="cst", bufs=1) as cst,
            tc.tile_pool(name="gp", bufs=int(_os.environ.get("GCN_GPB", "8"))) as gp,
            tc.tile_pool(name="sp", bufs=int(_os.environ.get("GCN_SPB", "4"))) as sp,
            tc.tile_pool(name="pm", bufs=4, space="PSUM") as pm,
        ):
            # load order: gidx first (gates gathers), then weights (gates
            # PE); binit + sidx issue after the first gathers (they gate
            # only the first scatter)
            gidx_t = cst.tile([128, nblk_total * 8], I16, tag="gidx")
            nc.sync.dma_start(out=gidx_t[:], in_=gidx[:])
            wsb_t = cst.tile([128, 2, 21, 2, D], F8E5, tag="wsb")
            nc.sync.dma_start(out=wsb_t[:], in_=wsb[:])

            # gather pieces of GB blocks, issued lazily (prefetch depth 2)
            n_gp = (nblk_total + GB - 1) // GB
            g_pieces = [None] * n_gp

            def issue_gather(j):
                if j >= n_gp or g_pieces[j] is not None:
                    return
                p0 = j * GB
                pn = min(GB, nblk_total - p0)
                # token (p, jrow, e) of the transpose-gather = fp8 pair
                # (x8[jrow*128+p], xr[jrow*128+p]) of edge e
                g_p = gp.tile([128, 2, pn * 128], I16, tag="g")
                nc.gpsimd.dma_gather(
                    out_ap=g_p[:],
                    in_ap=x_ext[:],
                    idxs_ap=gidx_t[:, p0 * 8:(p0 + pn) * 8],
                    num_idxs=pn * 128,
                    num_idxs_reg=pn * 128,
                    elem_size=D,
                    transpose=True,
                    queue_num=0,
                )
                g_pieces[j] = g_p

            issue_gather(0)
            issue_gather(1)

            # out init: DRAM->DRAM copies of the bias image (per half so
            # the halves' WAW chains stay independent)
            nc.sync.dma_start(out=out[0:HROWS, :], in_=binit[0:HROWS, :])
            nc.sync.dma_start(out=out[HROWS:2 * HROWS, :],
                              in_=binit[HROWS:2 * HROWS, :])
            sidx_t = cst.tile([128, nblk_total * 8], I16, tag="sidx")
            nc.sync.dma_start(out=sidx_t[:], in_=sidx[:])

            reps = int(_os.environ.get("GCN_REPS", "1"))
            copy_i = 0
            for _rep in range(reps):
              if _rep > 0:
                g_pieces[:] = [None] * n_gp
                issue_gather(0)
                issue_gather(1)
                nc.sync.dma_start(out=out[0:HROWS, :],
                                  in_=binit[0:HROWS, :])
                nc.sync.dma_start(out=out[HROWS:2 * HROWS, :],
                                  in_=binit[HROWS:2 * HROWS, :])
              bi = 0
              for (rw, h, nb) in pieces:
                  msg = sp.tile([128, nb, D], F16, tag="msg")
                  k = 0
                  while k < nb:
                      # pair up to 2 blocks in one PSUM bank -> one copy
                      kn = min(2, nb - k)
                      m_ps = pm.tile([128, 2, D], F32, tag="m")
                      for j in range(kn):
                          b = bi + k + j
                          if b % GB == 0:
                              issue_gather(b // GB + 2)
                          g_p = g_pieces[b // GB]
                          sub = b % GB
                          sl = slice(sub * 128, (sub + 1) * 128)
                          g8e5 = g_p[:].bitcast(F8E5).rearrange(
                              "p j (e two) -> p j e two", two=2)[:, :, sl, :]
                          g8e4 = g_p[:].bitcast(F8E4).rearrange(
                              "p j (e two) -> p j e two", two=2)[:, :, sl, :]
                          # msg = x8 @ W8 + x8 @ Wr + xr @ W8  (DoubleRow:
                          # dim1 of lhsT/rhs = the two 128-wide k-tiles;
                          # the xr term mixes e4m3 lhsT with the e5m2 rhs,
                          # verified exact on HW)
                          nc.tensor.matmul(
                              out=m_ps[:, j, :],
                              lhsT=g8e5[:, :, :, 0],
                              rhs=wsb_t[:, 0, rw, :, :],
                              perf_mode=DR,
                              start=True, stop=False)
                          nc.tensor.matmul(
                              out=m_ps[:, j, :],
                              lhsT=g8e5[:, :, :, 0],
                              rhs=wsb_t[:, 1, rw, :, :],
                              perf_mode=DR,
                              start=False, stop=False)
                          nc.tensor.matmul(
                              out=m_ps[:, j, :],
                              lhsT=g8e4[:, :, :, 1],
                              rhs=wsb_t[:, 0, rw, :, :],
                              perf_mode=DR,
                              start=False, stop=True)
                      if copy_i % 2 == 0:
                          nc.scalar.copy(out=msg[:, k:k + kn, :],
                                         in_=m_ps[:, 0:kn, :])
                      else:
                          nc.vector.tensor_copy(msg[:, k:k + kn, :],
                                                m_ps[:, 0:kn, :])
                      copy_i += 1
                      k += kn
                  nc.gpsimd.dma_scatter_add(
                      out_ap=out[h * HROWS:(h + 1) * HROWS, :],
                      in_ap=msg[:],
                      idxs_ap=sidx_t[:, bi * 8:(bi + nb) * 8],
                      num_idxs=nb * 128,
                      num_idxs_reg=nb * 128,
                      elem_size=D,
                      queue_num=1 + h,
                  )
                  bi += nb
    nc.finalize()
    return nc


# ---------------------------------------------------------------- entry

def kernel(x, W_self, b_self, W_fwd, b_fwd, W_rev, b_rev,
           dep_idx, rel_idx, gov_idx, _trace=False, _trace_kwargs=None):
    pieces, nblk_total, nx_total, in_maps = prepare(
        x, W_self, b_self, W_fwd, b_fwd, W_rev, b_rev,
        dep_idx, rel_idx, gov_idx)
    nc = build_bass(pieces, nblk_total, nx_total)
    res = run_bass_kernel_spmd(nc, in_maps, list(range(N_CORES)),
                               trace=_trace, **(_trace_kwargs or {}))
    outs = []
    for c in range(N_CORES):
        o = res.results[c]["out"]
        outs.append(o[0:HALF])
        outs.append(o[HROWS:HROWS + HALF])
    kernel._last_results = res
    return np.concatenate(outs, axis=0).astype(np.float32)

